# revision 53
# baseline (speedup 1.0000x reference)
"""Trainium2 Bass kernel for nn_Attention_30408368456170 — fp8 DoubleRow redesign.

Data-parallel over batch B=8 -> 8 cores. Per core:
  - qproj/conv1/conv2 in fp8-e4m3 DoubleRow (K-paired, 2x FLOP rate), weights
    prescaled x16 for fp8 range health (LN absorbs conv scale via eps*256;
    exp scale absorbs the q-side 16).
  - QK: per kv-block, 4 heads concurrently via 32-row tile_position groups,
    Ki=32 fp8-DR (head-dim pairs in the j slot).
  - exp over [128,4,512] psum (4 banks, 4 heads) in one ACT pass -> fp8 P in
    DR-paired layout.
  - PV: fused O+D matmul: lhsT = [v^T | ones] (even heads) or [ones | v^T]
    (odd heads), M=128 fp8-DR. O lands in the catT-half the head needs;
    D lands replicated in the other 64 partitions of the same bank.
  - softmax divide: per head-pair reciprocal + partition-aligned multiply
    into catT.
  - kv projections and final proj stay bf16 (fp8 proj measured 3.95%
    end-to-end vs the 2% gate); proj_b added on host.

Schedule (the perf-critical part): conv1+qproj, then br1 k/v/locconv prep,
then conv2 with ONLY br1 QK/exp woven in (conv groups hold all 4 psv psum
ring slots, so any PV/div piece woven there head-of-line blocks the PE
FIFO).  After conv2, one continuous exp-paced stream runs br2 nt0..7 with
pops draining br1's PV/div backlog, br2 PV/div, and proj pieces.  br1 nt0-3
alias Pp2a r-slices (their PV pops precede the nt1 exps that overwrite
them); br1 nt4-7 use dedicated Pp1a-d so Pp2b is WAR-free for br2-nt0 at
stream entry; br1-nt4-7 div chains are injected at nt1/nt3 so the DVE FIFO
near the entry stays clear for locconv2-band2 (emitted at nt0-mt6) -> tv2.
Scheduling hazard: a psv-pool tile allocation reuses the ring slot of the
4-back allocation and only orders against consumers emitted SO FAR — never
emit a psv alloc before all readers of that earlier tile are emitted (this
raced when tv1/xgT were woven into conv2-g0's fillers).
"""

import numpy as np
import ml_dtypes

import concourse.bass as bass
import concourse.mybir as mybir
import concourse.tile as tile
from concourse import bacc
from concourse.masks import make_identity

E4 = ml_dtypes.float8_e4m3fn
BFD = ml_dtypes.bfloat16
F32 = mybir.dt.float32
BF16 = mybir.dt.bfloat16
FP8 = mybir.dt.float8e4
AF = mybir.ActivationFunctionType
ALU = mybir.AluOpType
DR = mybir.MatmulPerfMode.DoubleRow

C = 512
N = 4096
HH = 64
WS = 16.0                 # fp8 weight prescale
EPS_S = 1e-5 * WS * WS    # LN eps on (16x)^2-scaled variance
EXPS = 0.125 / WS         # exp scale: head-dim softmax scale / weight prescale

BR1 = dict(ks=5, h=16, m=256, MT=2, MTP=1)
BR2 = dict(ks=3, h=32, m=1024, MT=8, MTP=4)

TRACE = False
DEBUG = False
LAST_RESULT = None


def _emit_stats(nc, stat_p, src, pt, var, mean):
    st = stat_p.tile([128, 6], F32, tag="st", name=f"st{pt}")
    nc.vector.bn_stats(out=st, in_=src)
    mv = stat_p.tile([128, 2], F32, tag="mv", name=f"mv{pt}")
    nc.vector.bn_aggr(out=mv, in_=st)
    nc.vector.tensor_copy(mean[:, pt:pt + 1], mv[:, 0:1])
    nc.vector.tensor_copy(var[:, pt:pt + 1], mv[:, 1:2])


def _emit_rs(nc, eps_sb, var, lnv, rs, mean, ba, p0, p1):
    # rs = 1/sqrt(var + eps) via DVE add + ONE Sqrt table func + DVE
    # reciprocal (the old Ln+Exp pair put two serial ACT-table loads on
    # every conv-boundary critical chain; Rsqrt is framework-blocked)
    nc.vector.tensor_scalar_add(out=lnv[:, p0:p1], in0=var[:, p0:p1],
                                scalar1=EPS_S)
    nc.scalar.activation(out=var[:, p0:p1], in_=lnv[:, p0:p1],
                         func=AF.Sqrt, scale=1.0)
    nc.vector.reciprocal(out=rs[:, p0:p1], in_=var[:, p0:p1])
    for pt in range(p0, p1):
        nc.vector.scalar_tensor_tensor(
            out=ba[:, pt:pt + 1], in0=mean[:, pt:pt + 1],
            scalar=-1.0, in1=rs[:, pt:pt + 1],
            op0=ALU.mult, op1=ALU.mult)


def _build():
    nc = bacc.Bacc("TRN2", target_bir_lowering=False)

    xt8_d = nc.dram_tensor("xt8", [128, 2, 2, N], FP8, kind="ExternalInput")
    qw8_d = nc.dram_tensor("qw8", [128, 2, 2, C], FP8, kind="ExternalInput")
    xim1_d = nc.dram_tensor("xim1", [128, 25, 2, 2, 256], FP8,
                            kind="ExternalInput")
    w1_d = nc.dram_tensor("w1", [128, 25, 2, 2, C], FP8, kind="ExternalInput")
    xim2_d = nc.dram_tensor("xim2", [128, 9, 2, 2, 1024], FP8,
                            kind="ExternalInput")
    w2_d = nc.dram_tensor("w2", [128, 9, 2, 2, C], FP8, kind="ExternalInput")
    kv1_d = nc.dram_tensor("kv1", [128, 4, C], BF16, kind="ExternalInput")
    kv2_d = nc.dram_tensor("kv2", [128, 4, C], BF16, kind="ExternalInput")
    pw_d = nc.dram_tensor("pw", [128, 4, C], BF16, kind="ExternalInput")
    lc1_d = nc.dram_tensor("lc1", [128, 2, 9], F32, kind="ExternalInput")
    lc2_d = nc.dram_tensor("lc2", [128, 2, 9], F32, kind="ExternalInput")
    out_d = nc.dram_tensor("out", [N, C], BF16, kind="ExternalOutput")
    if DEBUG:
        dbg_qT = nc.dram_tensor("dbg_qT", [128, 2, 2, N], FP8,
                                kind="ExternalOutput")
        dbg_kT1 = nc.dram_tensor("dbg_kT1", [128, 2, 256], FP8,
                                 kind="ExternalOutput")
        dbg_kT2 = nc.dram_tensor("dbg_kT2", [128, 2, 1024], FP8,
                                 kind="ExternalOutput")
        dbg_va1 = nc.dram_tensor("dbg_va1", [128, 4, 1, 2, 128], FP8,
                                 kind="ExternalOutput")
        dbg_va2 = nc.dram_tensor("dbg_va2", [128, 4, 4, 2, 128], FP8,
                                 kind="ExternalOutput")
        dbg_Pp1 = nc.dram_tensor("dbg_Pp1", [128, 4, 1, 2, 512], FP8,
                                 kind="ExternalOutput")
        dbg_cat = nc.dram_tensor("dbg_cat", [128, 4, N], BF16,
                                 kind="ExternalOutput")
        dbg_x1g = nc.dram_tensor("dbg_x1g", [128, 2, C], BF16,
                                 kind="ExternalOutput")
        dbg_ab = nc.dram_tensor("dbg_ab", [2, 2, 128, 512], F32,
                                kind="ExternalOutput")
        dbg_rsw = nc.dram_tensor("dbg_rsw", [2, 128, 512], F32,
                                 kind="ExternalOutput")

    with tile.TileContext(nc) as tc:
        with (
            tc.tile_pool(name="persist", bufs=1) as persist,
            tc.tile_pool(name="psqk", bufs=2, space="PSUM") as psqk,
            tc.tile_pool(name="psv", bufs=4, space="PSUM") as psv,
            tc.tile_pool(name="stat_pool", bufs=4) as stat_p,
            tc.tile_pool(name="mpool", bufs=2) as mpool,
            tc.tile_pool(name="outp", bufs=2) as outp,
            tc.tile_pool(name="dbp", bufs=4, space="DRAM") as drp,
        ):
            qT8 = persist.tile([128, 2, 2, N], FP8)
            catT = persist.tile([128, 4, N], BF16)
            kT1 = persist.tile([128, 2, 256], FP8)
            kT2 = persist.tile([128, 2, 1024], FP8)
            vaug1 = persist.tile([128, 4, 1, 2, 128], FP8)
            vaug2 = persist.tile([128, 4, 4, 2, 128], FP8)
            qw8 = persist.tile([128, 2, 2, C], FP8)
            kv1_sb = persist.tile([128, 4, C], BF16)
            kv2_sb = persist.tile([128, 4, C], BF16)
            pw_sb = persist.tile([128, 4, C], BF16)
            lc1_sb = persist.tile([128, 2, 9], F32)
            lc2_sb = persist.tile([128, 2, 9], F32)
            ident_bf = persist.tile([128, 128], BF16)
            make_identity(nc, ident_bf)
            eps_sb = persist.tile([128, 1], F32)
            nc.vector.memset(eps_sb, EPS_S)
            # ones halves of vaug produce the softmax denominators
            nc.gpsimd.memset(vaug1, 1.0)
            nc.gpsimd.memset(vaug2, 1.0)

            x1g = persist.tile([128, 2, C], BF16)
            x1gT = persist.tile([128, 4, 256], BF16)
            rs1 = persist.tile([128, 2], F32)
            ba1 = persist.tile([128, 2], F32)
            var1 = persist.tile([128, 2], F32)
            mean1 = persist.tile([128, 2], F32)
            lnv1 = persist.tile([128, 2], F32)
            rs2 = persist.tile([128, 8], F32)
            ba2 = persist.tile([128, 8], F32)
            var2 = persist.tile([128, 8], F32)
            mean2 = persist.tile([128, 8], F32)
            lnv2 = persist.tile([128, 8], F32)

            Pp1a = persist.tile([128, 4, 1, 2, 512], FP8)
            Pp1b = persist.tile([128, 4, 1, 2, 512], FP8)
            Pp1c = persist.tile([128, 4, 1, 2, 512], FP8)
            Pp1d = persist.tile([128, 4, 1, 2, 512], FP8)
            Pp2a = persist.tile([128, 4, 4, 2, 512], FP8)
            Pp2b = persist.tile([128, 4, 4, 2, 512], FP8)
            # br1 nt0-3 alias Pp2a r-slices (overwritten only by the br2
            # stream's nt1 exps, after their PV pops); nt4-7 get dedicated
            # buffers so Pp2b is clean for br2-nt0 at stream start
            Pp1list = [Pp2a[:, :, r:r + 1, :, :] for r in range(4)] + \
                [Pp1a, Pp1b, Pp1c, Pp1d]

            # ---------------- helpers ----------------

            def emit_qk_exp(br, nt, mt, kT, Pp):
                # two half-slots of 2 heads each; the 2-bank double-buffered
                # psqk ring lets the next half's QK stream during this exp
                for hp in range(2):
                    qk = psqk.tile([128, 2, 512], F32, tag="qk", name="qk")
                    for hi in range(2):
                        h = 2 * hp + hi
                        nc.tensor.matmul(
                            qk[:, hi, :],
                            lhsT=kT[32 * h:32 * h + 32, :,
                                    mt * 128:(mt + 1) * 128],
                            rhs=qT8[32 * h:32 * h + 32, br - 1, :,
                                    nt * 512:(nt + 1) * 512],
                            start=True, stop=True, perf_mode=DR,
                            tile_position=(32 * h, 0))
                    nc.scalar.activation(
                        out=Pp[:, 2 * hp:2 * hp + 2, mt // 2, mt % 2, :],
                        in_=qk, func=AF.Exp, scale=EXPS)

            def make_pv_piece(br, nt, hp, vaug, Pp, MTP):
                def piece():
                    tiles = []
                    for i in range(2):
                        h = 2 * hp + i
                        t = psv.tile([128, 512], F32, tag="pv", name=f"o{h}")
                        for r in range(MTP):
                            nc.tensor.matmul(
                                t, lhsT=vaug[:, h, r, :, :],
                                rhs=Pp[:, h, r, :, :],
                                start=(r == 0), stop=(r == MTP - 1),
                                perf_mode=DR)
                        tiles.append(t)
                    return tiles
                return piece

            def make_div_piece(br, nt, hp, tiles_box):
                def piece():
                    # Both heads' PV banks: D replicated at rows 0-63,
                    # O at rows 64-127. Partial reciprocal_approx_fast only
                    # works at partition base 0; tensor_mul handles mixed
                    # bases fine.
                    A, B = tiles_box[0]
                    rTA = mpool.tile([64, 512], F32, tag="rT", name="rTA")
                    rTB = mpool.tile([64, 512], F32, tag="rT", name="rTB")
                    nc.vector.reciprocal_approx_fast(out=rTA, in_=A[0:64, :])
                    nc.vector.reciprocal_approx_fast(out=rTB, in_=B[0:64, :])
                    ct = 2 * (br - 1) + hp
                    sl = catT[:, ct, nt * 512:(nt + 1) * 512]
                    nc.vector.tensor_mul(out=sl[0:64, :], in0=A[64:128, :],
                                         in1=rTA)
                    nc.vector.tensor_mul(out=sl[64:128, :], in0=B[64:128, :],
                                         in1=rTB)
                    if DEBUG and hp == 0 and ((br == 1 and nt == 0)
                                              or (br == 2 and nt == 5)):
                        di = 0 if br == 1 else 1
                        ac = mpool.tile([128, 512], F32, tag="dbgc",
                                        name="dbgc", bufs=1)
                        nc.vector.tensor_copy(ac, A)
                        nc.sync.dma_start(dbg_ab[di, 0], ac)
                        bc = mpool.tile([128, 512], F32, tag="dbgc",
                                        name="dbgc2", bufs=1)
                        nc.vector.tensor_copy(bc, B)
                        nc.sync.dma_start(dbg_ab[di, 1], bc)
                        nc.sync.dma_start(dbg_rsw[di], rSW)
                return piece

            def make_attn_pieces(br, nt):
                """PV+divide pieces for chunk (br, nt); Pp must be complete."""
                p = BR1 if br == 1 else BR2
                vaug = vaug1 if br == 1 else vaug2
                if br == 1:
                    Pp = Pp1list[nt]
                else:
                    Pp = Pp2b if nt % 2 == 0 else Pp2a
                pieces = []
                for hp in range(2):
                    box = [None]
                    pv = make_pv_piece(br, nt, hp, vaug, Pp, p["MT"] // 2)

                    def pvwrap(pv=pv, box=box):
                        box[0] = pv()
                    pieces.append(pvwrap)
                    pieces.append(make_div_piece(br, nt, hp, box))
                return pieces

            def make_proj_pieces(nt):
                pieces = []
                for sub in range(4):
                    def piece(sub=sub):
                        nt32 = nt * 4 + sub
                        acc = psv.tile([128, 512], F32, tag="pv", name="pj")
                        for ci in range(4):
                            nc.tensor.matmul(
                                acc,
                                lhsT=catT[:, ci, nt32 * 128:(nt32 + 1) * 128],
                                rhs=pw_sb[:, ci, :],
                                start=(ci == 0), stop=(ci == 3))
                        ob = outp.tile([128, 512], BF16, tag="ob",
                                       name="ob")
                        nc.vector.tensor_copy(ob, acc)
                        nc.sync.dma_start(
                            out_d[nt32 * 128:(nt32 + 1) * 128, :], ob)
                    pieces.append(piece)
                return pieces

            def emit_conv_group(xim_d, w_d, br, group, wpool, tag,
                                filler=None, tb=3):
                """Conv group with tap-BLOCK DMAs (tb taps per DMA pair)."""
                p = BR1 if br == 1 else BR2
                ks, m = p["ks"], p["m"]
                ntap = ks * ks
                gp = len(group) * 128
                p0 = group[0] * 128
                psums = [psv.tile([128, 512], F32, tag="pv",
                                  name=f"cv{tag}{pt}") for pt in group]
                blocks = [range(t0, min(t0 + tb, ntap))
                          for t0 in range(0, ntap, tb)]
                for blk in blocks:
                    nb = len(blk)
                    t0 = blk[0]
                    wt = wpool.tile([128, tb, 2, 2, C], FP8, tag=f"wt{br}",
                                    name=f"wt{tag}{t0}", bufs=2)
                    nc.sync.dma_start(wt[:, :nb], w_d[:, t0:t0 + nb])
                    xt = wpool.tile([128, tb, 2, 2, gp], FP8, tag=f"xt{br}",
                                    name=f"xt{tag}{t0}", bufs=2)
                    nc.sync.dma_start(
                        xt[:, :nb],
                        xim_d[:, t0:t0 + nb, :, :, p0:p0 + gp])
                    for ti, tap in enumerate(blk):
                        for gi, pt in enumerate(group):
                            for cip in range(2):
                                nc.tensor.matmul(
                                    psums[gi],
                                    lhsT=xt[:, ti, cip, :,
                                            gi * 128:(gi + 1) * 128],
                                    rhs=wt[:, ti, cip, :, :],
                                    start=(tap == 0 and cip == 0),
                                    stop=(tap == ntap - 1 and cip == 1),
                                    perf_mode=DR)
                        if filler is not None:
                            filler(tap)
                return psums

            def emit_xgT(xg, xgT, pts):
                for pt in pts:
                    for ci in range(4):
                        tp = psv.tile([128, 512], BF16, tag="pv", name="tx")
                        nc.tensor.transpose(tp[:, 0:128],
                                            xg[:, pt, ci * 128:(ci + 1) * 128],
                                            ident_bf)
                        nc.vector.tensor_copy(
                            xgT[:, ci, pt * 128:(pt + 1) * 128], tp[:, 0:128])

            def emit_kv_chunk(kv_sb, kT8, xgT, vsrc, ch, csz, parts="kv"):
                for kb in (range(2) if "k" in parts else ()):
                    acc = psv.tile([128, 512], F32, tag="pv", name="kv")
                    for ci in range(4):
                        nc.tensor.matmul(
                            acc[:, :csz],
                            lhsT=kv_sb[:, ci, kb * 128:(kb + 1) * 128],
                            rhs=xgT[:, ci, ch * 512:ch * 512 + csz],
                            start=(ci == 0), stop=(ci == 3))
                    nc.vector.tensor_copy(
                        kT8[:, kb, ch * 512:ch * 512 + csz], acc[:, :csz])
                for vt in (range(2) if "v" in parts else ()):
                    acc = psv.tile([128, 512], F32, tag="pv", name="vv")
                    for ci in range(4):
                        nc.tensor.matmul(
                            acc[:, :csz],
                            lhsT=kv_sb[:, ci, 256 + vt * 128:
                                       256 + (vt + 1) * 128],
                            rhs=xgT[:, ci, ch * 512:ch * 512 + csz],
                            start=(ci == 0), stop=(ci == 3))
                    nc.vector.tensor_copy(
                        vsrc[:, vt, ch * 512:ch * 512 + csz], acc[:, :csz])

            def emit_locconv_band(lc_sb, vsrc, vacc, h, r0, r1):
                # bf16 SBUF->SBUF copy hits the 4x DVE mode — cheap there
                nc.vector.tensor_copy(vacc[:, :, r0 * h:r1 * h],
                                      vsrc[:, :, r0 * h:r1 * h])
                vs_img = vsrc.rearrange("p t (h w) -> p t h w", h=h)
                va_img = vacc.rearrange("p t (h w) -> p t h w", h=h)
                for tap in range(9):
                    dy, dx = tap // 3 - 1, tap % 3 - 1
                    ys, ye = max(0, -dy), h - max(0, dy)
                    xs, xe = max(0, -dx), h - max(0, dx)
                    oys, oye = max(ys, r0), min(ye, r1)
                    if oye <= oys:
                        continue
                    for vt in range(2):
                        nc.vector.scalar_tensor_tensor(
                            out=va_img[:, vt, oys:oye, xs:xe],
                            in0=vs_img[:, vt, oys + dy:oye + dy,
                                       xs + dx:xe + dx],
                            scalar=lc_sb[:, vt, tap:tap + 1],
                            in1=va_img[:, vt, oys:oye, xs:xe],
                            op0=ALU.mult, op1=ALU.add)

            def emit_locconv_tv(lc_sb, vsrc, vacc, vaug, h, MT):
                emit_locconv_band(lc_sb, vsrc, vacc, h, 0, h)
                emit_tv(vacc, vaug, MT)

            def emit_tv(vacc, vaug, MT, vts=(0, 1)):
                # one 128x128 transpose serves BOTH heads of a vt (they are
                # the two channel halves of the transposed token chunk)
                for vt in vts:
                    for mt in range(MT):
                        tp = psv.tile([128, 512], BF16, tag="pv", name="tv")
                        nc.tensor.transpose(
                            tp[:, 0:128],
                            vacc[:, vt, mt * 128:(mt + 1) * 128],
                            ident_bf)
                        for half in range(2):
                            nc.vector.tensor_copy(
                                vaug[:, 2 * vt + half, mt // 2, mt % 2,
                                     64:128],
                                tp[:, half * 64:half * 64 + 64])

            # ------- phase 1+2: DMAs, conv1 with qproj woven in, prep1 ----
            with tc.tile_pool(name="xpool", bufs=1) as xp:
                xt8_sb = xp.tile([128, 2, 2, N], FP8)
                # issue on the Pool queue so the Sync queue leads with the
                # conv1 wt/xt block DMAs (critical path to first matmul)
                nc.gpsimd.dma_start(qw8, qw8_d[:])
                nc.gpsimd.dma_start(xt8_sb[:, :, :, 0:1024],
                                    xt8_d[:, :, :, 0:1024])

                qp_jobs = [(ofb, ch) for ch in range(8) for ofb in range(4)]

                def emit_qp(ofb, ch):
                    br_i, jq = divmod(ofb, 2)
                    acc = psv.tile([128, 512], F32, tag="pv", name="qp")
                    for cip in range(2):
                        nc.tensor.matmul(
                            acc,
                            lhsT=qw8[:, cip, :, ofb * 128:(ofb + 1) * 128],
                            rhs=xt8_sb[:, cip, :, ch * 512:(ch + 1) * 512],
                            start=(cip == 0), stop=(cip == 1),
                            perf_mode=DR)
                    nc.vector.tensor_copy(
                        qT8[:, br_i, jq, ch * 512:(ch + 1) * 512], acc)

                def qp_filler(tap):
                    if tap in (2, 4, 6):
                        chg = tap // 2
                        nc.gpsimd.dma_start(
                            xt8_sb[:, :, :, chg * 1024:(chg + 1) * 1024],
                            xt8_d[:, :, :, chg * 1024:(chg + 1) * 1024])
                    elif tap == 8:
                        nc.gpsimd.dma_start(kv1_sb, kv1_d[:])
                        nc.gpsimd.dma_start(kv2_sb, kv2_d[:])
                    elif tap == 10:
                        nc.gpsimd.dma_start(pw_sb, pw_d[:])
                        nc.gpsimd.dma_start(lc1_sb, lc1_d[:])
                        nc.gpsimd.dma_start(lc2_sb, lc2_d[:])
                    if tap >= 3:
                        for _ in range(2):
                            if qp_jobs:
                                emit_qp(*qp_jobs.pop(0))

                with (
                    tc.tile_pool(name="wstream", bufs=6) as wpool,
                    tc.tile_pool(name="vwork1", bufs=1) as vw1,
                ):
                    cv1 = emit_conv_group(xim1_d, w1_d, 1, [0, 1], wpool,
                                          "a", filler=qp_filler)
                    for pt in range(2):
                        _emit_stats(nc, stat_p, cv1[pt], pt, var1, mean1)
                    _emit_rs(nc, eps_sb, var1, lnv1, rs1, mean1, ba1, 0, 2)
                    for pt in range(2):
                        nc.scalar.activation(
                            out=x1g[:, pt, :], in_=cv1[pt], func=AF.Gelu,
                            scale=rs1[:, pt:pt + 1], bias=ba1[:, pt:pt + 1])
                    while qp_jobs:
                        emit_qp(*qp_jobs.pop(0))
                    vsrc1 = vw1.tile([128, 2, 256], BF16)
                    vacc1 = vw1.tile([128, 2, 256], BF16)
                    emit_xgT(x1g, x1gT, range(2))
                    emit_kv_chunk(kv1_sb, kT1, x1gT, vsrc1, 0, 256,
                                  parts="kv")
                    # br1 v-prep right here: the psv ring is free (no conv
                    # group holds it) so locconv1 never jams a boundary;
                    # tv1 transposes are deferred into conv2-g0's fillers
                    # (<=3 parked at a time) so they don't head-of-line
                    # block conv2-g0 behind the locconv1 Vector chain
                    emit_locconv_band(lc1_sb, vsrc1, vacc1, 16, 0, 16)

                    # ------- phase 3: conv2 groups + woven br1 QK/exp ----
                    with tc.tile_pool(name="x2pool", bufs=1) as x2p:
                        x2g = x2p.tile([128, 8, C], BF16)
                        x2gT = x2p.tile([128, 4, 1024], BF16)
                        pending = []

                        br1_slots = [(nt, mt) for nt in range(8)
                                     for mt in range(2)]

                        def br1_slot():
                            if not br1_slots:
                                return
                            nt, mt = br1_slots.pop(0)
                            emit_qk_exp(1, nt, mt, kT1, Pp1list[nt])
                            if mt == 1:
                                pending.extend(make_attn_pieces(1, nt))

                        fill_ci = [0]

                        def br1_filler(tap):
                            # every other tap: parked QKs (psqk ring waits
                            # the exp pipeline) would stall conv matmuls
                            # behind them in the 4-deep PE wait queue
                            ci = fill_ci[0]
                            fill_ci[0] += 1
                            if ci % 2 == 0:
                                br1_slot()

                        with tc.tile_pool(name="vwork2", bufs=1) as vw2:
                            vsrc2 = vw2.tile([128, 2, 1024], BF16)
                            vacc2 = vw2.tile([128, 2, 1024], BF16)
                            # conv2: NO pops woven — conv psums hold the
                            # whole psv ring, so pieces would head-of-line
                            # block the PE FIFO. br1 QK/exp only (psqk).
                            for g in range(2):
                                pts = [4 * g + i for i in range(4)]
                                cv2 = emit_conv_group(xim2_d, w2_d, 2, pts,
                                                      wpool, f"b{g}",
                                                      filler=br1_filler,
                                                      tb=2)
                                for gi, pt in enumerate(pts):
                                    _emit_stats(nc, stat_p, cv2[gi], pt,
                                                var2, mean2)
                                _emit_rs(nc, eps_sb, var2, lnv2, rs2, mean2,
                                         ba2, 4 * g, 4 * g + 4)
                                for gi, pt in enumerate(pts):
                                    nc.scalar.activation(
                                        out=x2g[:, pt, :], in_=cv2[gi],
                                        func=AF.Gelu, scale=rs2[:, pt:pt + 1],
                                        bias=ba2[:, pt:pt + 1])
                                if g == 0:
                                    # tv1 here: vacc1 long settled, all cv
                                    # psum consumers emitted (ring-safe)
                                    emit_tv(vacc1, vaug1, 2)
                                emit_xgT(x2g, x2gT, pts)
                                br1_slot()
                                br1_slot()
                                if g == 0:
                                    emit_kv_chunk(kv2_sb, kT2, x2gT, vsrc2,
                                                  0, 512)
                                    br1_slot()
                                    # locconv2 rows 0-14 only read src rows
                                    # <16 = vsrc2 cols <512, all from g0
                                    emit_locconv_band(lc2_sb, vsrc2, vacc2,
                                                      32, 0, 15)
                                    # br2-nt0 mt0/mt1 early: kT2 cols 0-255
                                    # ready, Pp2b WAR-free; fills the g0->g1
                                    # Scalar gap (only 4 QKs park at most)
                                    for mt in range(2):
                                        emit_qk_exp(2, 0, mt, kT2, Pp2b)
                                else:
                                    emit_kv_chunk(kv2_sb, kT2, x2gT, vsrc2,
                                                  1, 512, parts="v")
                                    emit_kv_chunk(kv2_sb, kT2, x2gT, vsrc2,
                                                  1, 512, parts="k")
                            while br1_slots:
                                br1_slot()
                            # defer br1-nt4-7 pieces (dedicated P buffers,
                            # catT needed only by late proj) so their div
                            # chains don't flood the DVE at stream entry
                            br1_late = pending[16:]
                            del pending[16:]

                            # ------- phase 5: unified attn+proj stream ----
                            # br1's 32 pieces drain via pops paced by the
                            # br2 exp stream; emission order keeps the
                            # Pp2a/b alias WAR correct (PV(br1-nt) pops
                            # before the exp that overwrites its P slice).
                            for nt in range(8):
                                # nt0 -> Pp2b (WAR-free at stream start)
                                Pp = Pp2b if nt % 2 == 0 else Pp2a
                                if nt == 1:
                                    # two QKs ahead of tv2 in the PE FIFO
                                    # keep Scalar fed while tv2 waits on
                                    # band2; tv2 still precedes the
                                    # PV(br2-nt0) pops that read vaug2
                                    emit_qk_exp(2, 1, 0, kT2, Pp)
                                    emit_qk_exp(2, 1, 1, kT2, Pp)
                                    emit_tv(vacc2, vaug2, 8)
                                for _ in range(4):
                                    if pending:
                                        pending.pop(0)()
                                if nt > 0:
                                    pending.extend(make_proj_pieces(nt - 1))
                                if nt == 1:
                                    pending.extend(br1_late[:8])
                                elif nt == 3:
                                    pending.extend(br1_late[8:])
                                for mt in (range(2, 8) if nt <= 1
                                           else range(8)):
                                    emit_qk_exp(2, nt, mt, kT2, Pp)
                                    # nt>=2: 1 pop/mt spreads the PV/proj
                                    # PE bursts across the whole block
                                    for _ in range(3 if nt < 2 else 1):
                                        if pending:
                                            pending.pop(0)()
                                    if nt == 0 and mt == 6:
                                        # band2 behind all 8 br1-nt0-3 div
                                        # chains (they pace the psv ring)
                                        # but early enough for tv2 -> PV
                                        emit_locconv_band(lc2_sb, vsrc2,
                                                          vacc2, 32, 15, 32)
                                pending.extend(make_attn_pieces(2, nt))
                            while pending:
                                pending.pop(0)()
                            for piece in make_proj_pieces(7):
                                piece()

                        if DEBUG:
                            nc.sync.dma_start(dbg_qT[:], qT8)
                            nc.sync.dma_start(dbg_kT1[:], kT1)
                            nc.sync.dma_start(dbg_kT2[:], kT2)
                            nc.sync.dma_start(dbg_va1[:], vaug1)
                            nc.sync.dma_start(dbg_va2[:], vaug2)
                            nc.sync.dma_start(dbg_Pp1[:], Pp1b)
                            nc.sync.dma_start(dbg_cat[:], catT)
                            nc.sync.dma_start(dbg_x1g[:], x1g)

    nc.finalize()
    return nc


# ============================ host side ============================

def _q8(a):
    return np.clip(np.asarray(a, np.float32), -240.0, 240.0).astype(E4)


def _fold8(a2d):
    """[512 rows, F] -> [128, 2cip, 2j, F] with row f = 128*(2cip+j)+p."""
    F = a2d.shape[1]
    return np.ascontiguousarray(
        a2d.reshape(2, 2, 128, F).transpose(2, 0, 1, 3))


def _part_fold(a):
    return np.ascontiguousarray(a.reshape(4, 128, -1).transpose(1, 0, 2))


def _prep_shared(inputs):
    gi = lambda k: np.asarray(inputs[k], np.float32)
    shared = {}
    # qw8: column permutation for head/slot layout + x16 prescale
    perm = np.empty(512, np.int64)
    p = np.arange(128)
    for ofb in range(4):
        br_i, jq = divmod(ofb, 2)
        perm[128 * ofb + p] = 64 * (4 * br_i + p // 32) + (p % 32) + 32 * jq
    shared["qw8"] = _fold8(_q8(gi("q_w")[:, perm] * WS).astype(np.float32)
                           ).astype(E4)
    # conv weights: [taps, in 512, out 512] * 16 -> fold rows, tap-inner
    for name, key, ks in (("w1", "sr1_w", 5), ("w2", "sr2_w", 3)):
        w = np.transpose(gi(key), (2, 3, 1, 0)).reshape(ks * ks, C, C) * WS
        arr = np.stack([_fold8(_q8(w[t]).astype(np.float32)).astype(E4)
                        for t in range(ks * ks)])        # [t,128,2,2,C]
        shared[name] = np.ascontiguousarray(arr.transpose(1, 0, 2, 3, 4))
    # kv: permute k-half columns into (j, head, c%32) blocks; bf16
    kperm = np.empty(512, np.int64)
    for jk in range(2):
        kperm[128 * jk + p] = 64 * (p // 32) + (p % 32) + 32 * jk
    kperm[256:] = np.arange(256, 512)
    for name, key in (("kv1", "kv1_w"), ("kv2", "kv2_w")):
        shared[name] = _part_fold(gi(key)[:, kperm].astype(BFD))
    shared["pw"] = _part_fold(gi("proj_w").astype(BFD))
    for name, key in (("lc1", "lc1_w"), ("lc2", "lc2_w")):
        lcw = gi(key).reshape(256, 9)
        rows = np.arange(256)
        head, a, cp = rows // 64, (rows % 64) // 32, rows % 32
        w_rows = lcw[a * 128 + cp * 4 + head]
        shared[name] = np.ascontiguousarray(
            w_rows.reshape(2, 128, 9).transpose(1, 0, 2).astype(np.float32))
    return shared


def _prep_x(xb_f32):
    x8 = _q8(xb_f32.T)                                   # [C, N] fp8
    x8f = x8.astype(np.float32)
    m = {"xt8": _fold8(x8f).astype(E4)}
    img = x8f.reshape(C, HH, HH)
    pad = np.zeros((C, HH + 2, HH + 2), np.float32)
    pad[:, 1:HH + 1, 1:HH + 1] = img
    for name, br, stride in (("xim1", BR1, 4), ("xim2", BR2, 2)):
        ks, h = br["ks"], br["h"]
        span = stride * (h - 1) + 1
        im = np.empty((ks * ks, 128, 2, 2, h * h), E4)
        for tap in range(ks * ks):
            di, dj = tap // ks, tap % ks
            sl = pad[:, di:di + span:stride,
                     dj:dj + span:stride].reshape(C, h * h)
            im[tap] = _fold8(sl).astype(E4)
        m[name] = np.ascontiguousarray(im.transpose(1, 0, 2, 3, 4))
    return m


def kernel(**inputs):
    global LAST_RESULT
    from concourse.bass_utils import run_bass_kernel_spmd

    x = np.asarray(inputs["x"], np.float32)
    B = x.shape[0]
    assert B == 8 and x.shape[1] == N and x.shape[2] == C
    assert int(inputs["H"]) == HH and int(inputs["W"]) == HH
    for zkey in ("sr1_b", "sr2_b", "norm1_b", "norm2_b", "lc1_b", "lc2_b"):
        assert not np.any(np.asarray(inputs[zkey])), f"{zkey} expected zero"
    for okey in ("norm1_w", "norm2_w"):
        assert np.all(np.asarray(inputs[okey]) == 1.0), f"{okey} expected ones"

    shared = _prep_shared(inputs)
    in_maps = []
    for b in range(B):
        mm = dict(shared)
        mm.update(_prep_x(x[b]))
        in_maps.append(mm)

    nc = _build()
    res = run_bass_kernel_spmd(nc, in_maps, core_ids=list(range(8)),
                               trace=TRACE)
    LAST_RESULT = res
    out = np.stack([res.results[b]["out"].astype(np.float32)
                    for b in range(B)])
    out = out + np.asarray(inputs["proj_b"], np.float32)[None, None, :]
    return out.astype(np.float32)

